# revision 12
# baseline (speedup 1.0000x reference)
"""Distributed causal multi-head attention for TRN2, 8 NeuronCores.

Sharding: core c (0..7) handles batch c//4 and heads 4*(c%4)..4*(c%4)+3
(tensor-parallel over heads x data-parallel over batch).

Per-core pipeline (all matmuls bf16, fp32 PSUM accumulate):
  1. QKV projections from host-pretransposed xT:
       QT/KT[k,s] = (W.T x.T) with W tiles stationary;
       V[s,k] with xT tiles stationary (4 heads packed in the free dim).
  2. Attention per head, scores transposed: ST[s,q] = KT.T @ QT.
     exp on ACT, causal masking by precomputed 0/1 tiles, then
     z[q,k] and the softmax row-sum r[q] in ONE matmul per
     (q-tile, s-tile): rhs = [V | ones] (129 columns).
     Normalize z by 1/r (per-partition scalar), PE-transpose to zT and
     DMA to the AllGather buffer in [hk_local, q] layout.
  3. AllGather over the 4-core batch group: every core gets the full
     zT [16*128, 2048] for its batch.
  4. Output projection, d-sharded: each core's wo input holds only its
     512 W_O columns, so out[all q, d_slice] = z_flat @ W_O[:, slice].
     The graph is identical on all cores; per-core behavior comes only
     from input data (SPMD-safe).

Host: shards/casts/transposes inputs, adds bias corrections
(b_O + sum_h b_V[h] @ W_O[h] is a constant row because softmax rows sum
to 1; b_Q/b_K are folded into the QT/KT PSUM evacuation on device).
"""
import math
import os

import numpy as np
import ml_dtypes

import concourse.bacc as bacc
import concourse.mybir as mybir
from concourse import tile, masks
from concourse.bass_utils import run_bass_kernel_spmd

BF16 = mybir.dt.bfloat16
F32 = mybir.dt.float32
NPBF16 = ml_dtypes.bfloat16

B = 2
SEQ = 2048
D_MODEL = 2048
N_HEADS = 16
D_HEAD = 128
HPC = 4              # heads per core
NCORES = 8
GROUPS = [[0, 1, 2, 3], [4, 5, 6, 7]]
NDT = D_MODEL // 128   # 16 d-model tiles
NST = SEQ // 128       # 16 seq tiles
NQC = SEQ // 512       # 4 q-chunks
QSL = SEQ // 4         # 512 per-core q-slice for output projection
SCALE = 1.0 / math.sqrt(D_HEAD)

LAST_EXEC_NS = None


def build_nc():
    nc = bacc.Bacc(None, num_devices=NCORES, debug=False)

    xt_e = nc.declare_dram_parameter("xt", [D_MODEL, SEQ], BF16, isOutput=False)
    wq_e = nc.declare_dram_parameter("wq", [HPC * D_MODEL, D_HEAD], BF16, isOutput=False)
    wk_e = nc.declare_dram_parameter("wk", [HPC * D_MODEL, D_HEAD], BF16, isOutput=False)
    wv_e = nc.declare_dram_parameter("wv", [D_MODEL, HPC * D_HEAD], BF16, isOutput=False)
    wo_e = nc.declare_dram_parameter("wo", [N_HEADS * D_HEAD, QSL], BF16, isOutput=False)
    bq_e = nc.declare_dram_parameter("bq", [D_HEAD, HPC], F32, isOutput=False)
    bk_e = nc.declare_dram_parameter("bk", [D_HEAD, HPC], F32, isOutput=False)
    mk_e = nc.declare_dram_parameter("mk", [128, 4 * 512], BF16, isOutput=False)
    out_e = nc.declare_dram_parameter("out", [SEQ, QSL], F32, isOutput=True)

    agin = nc.dram_tensor("agin", [HPC * D_HEAD, SEQ], BF16)
    agout = nc.dram_tensor("agout", [N_HEADS * D_HEAD, SEQ], BF16)

    with tile.TileContext(nc) as tc:
        with tc.tile_pool(name="persist", bufs=1) as pp, \
             tc.tile_pool(name="qkvout", bufs=4) as qk_pool:
            ident = pp.tile([128, 128], BF16, tag="ident")
            masks.make_identity(nc, ident[:])
            bq_sb = pp.tile([128, HPC], F32, tag="bq")
            nc.sync.dma_start(bq_sb[:], bq_e[:, :])
            bk_sb = pp.tile([128, HPC], F32, tag="bk")
            nc.sync.dma_start(bk_sb[:], bk_e[:, :])
            mk_sb = pp.tile([128, 4 * 512], BF16, tag="mk")
            nc.sync.dma_start(mk_sb[:], mk_e[:, :])

            qt_sb = [qk_pool.tile([128, SEQ], BF16, tag="qt", name=f"qt{h}")
                     for h in range(HPC)]
            kt_sb = [qk_pool.tile([128, SEQ], BF16, tag="kt", name=f"kt{h}")
                     for h in range(HPC)]
            v_sb = [qk_pool.tile([128, NST, D_HEAD + 1], BF16, tag="v",
                                 name=f"v{h}") for h in range(HPC)]
            for h in range(HPC):
                nc.vector.memset(v_sb[h][:, :, D_HEAD:D_HEAD + 1], 1.0)

            # ---------------- Phase 1: QKV projections ----------------
            with tc.tile_pool(name="xtp", bufs=NDT) as xt_pool, \
                 tc.tile_pool(name="wp", bufs=4) as w_pool, \
                 tc.tile_pool(name="ps1", bufs=3, space="PSUM") as ps1:
                xt_sb = []
                for dt in range(NDT):
                    t = xt_pool.tile([128, SEQ], BF16, tag="xt")
                    nc.sync.dma_start(t[:], xt_e[dt * 128:(dt + 1) * 128, :])
                    xt_sb.append(t)
                wq_sb, wk_sb = [], []
                for h in range(HPC):
                    tq = w_pool.tile([128, NDT, D_HEAD], BF16, tag="wq")
                    nc.sync.dma_start(
                        tq[:],
                        wq_e[h * D_MODEL:(h + 1) * D_MODEL, :]
                        .rearrange("(t p) k -> p t k", p=128))
                    wq_sb.append(tq)
                    tk = w_pool.tile([128, NDT, D_HEAD], BF16, tag="wk")
                    nc.sync.dma_start(
                        tk[:],
                        wk_e[h * D_MODEL:(h + 1) * D_MODEL, :]
                        .rearrange("(t p) k -> p t k", p=128))
                    wk_sb.append(tk)
                wv_sb = w_pool.tile([128, NDT, HPC * D_HEAD], BF16, tag="wv",
                                    bufs=1)
                nc.sync.dma_start(
                    wv_sb[:], wv_e.ap().rearrange("(t p) k -> p t k", p=128))

                # QT / KT: [k, s] per head
                for h in range(HPC):
                    for proj in range(2):
                        w_t = wq_sb[h] if proj == 0 else wk_sb[h]
                        dst = qt_sb[h] if proj == 0 else kt_sb[h]
                        for sc in range(NQC):
                            psum = ps1.tile([128, 512], F32, tag="ps1")
                            for dt in range(NDT):
                                nc.tensor.matmul(
                                    psum[:],
                                    w_t[:, dt, :],
                                    xt_sb[dt][:, sc * 512:(sc + 1) * 512],
                                    start=(dt == 0), stop=(dt == NDT - 1))
                            if proj == 0:
                                nc.scalar.activation(
                                    dst[:, sc * 512:(sc + 1) * 512], psum[:],
                                    mybir.ActivationFunctionType.Identity,
                                    bias=bq_sb[:, h:h + 1], scale=SCALE)
                            else:
                                nc.scalar.activation(
                                    dst[:, sc * 512:(sc + 1) * 512], psum[:],
                                    mybir.ActivationFunctionType.Identity,
                                    bias=bk_sb[:, h:h + 1], scale=1.0)

                # V: [s, k] with 4 heads packed
                for st in range(NST):
                    psum = ps1.tile([128, 512], F32, tag="ps1")
                    for dt in range(NDT):
                        nc.tensor.matmul(
                            psum[:],
                            xt_sb[dt][:, st * 128:(st + 1) * 128],
                            wv_sb[:, dt, :],
                            start=(dt == 0), stop=(dt == NDT - 1))
                    for h in range(HPC):
                        nc.scalar.copy(
                            v_sb[h][:, st, 0:D_HEAD],
                            psum[:, h * 128:(h + 1) * 128])

            # ---------------- Phase 2: attention per head ----------------
            with tc.tile_pool(name="pt", bufs=3) as pt_pool, \
                 tc.tile_pool(name="zz", bufs=3) as z_pool, \
                 tc.tile_pool(name="ps_st", bufs=2, space="PSUM") as ps_st, \
                 tc.tile_pool(name="ps_av", bufs=4, space="PSUM") as ps_av, \
                 tc.tile_pool(name="ps_tr", bufs=2, space="PSUM") as ps_tr:
                for h in range(HPC):
                    for j in range(NQC):
                        n_st = 4 * (j + 1)
                        avp = [ps_av.tile([128, D_HEAD + 1], F32, tag="av",
                                          name=f"av{h}_{j}_{t}") for t in range(4)]
                        for i in range(n_st):
                            stp = ps_st.tile([128, 512], F32, tag="st")
                            nc.tensor.matmul(
                                stp[:],
                                kt_sb[h][:, i * 128:(i + 1) * 128],
                                qt_sb[h][:, j * 512:(j + 1) * 512],
                                start=True, stop=True)
                            pt = pt_pool.tile([128, 512], BF16, tag="pt")
                            nc.scalar.activation(
                                pt[:], stp[:], mybir.ActivationFunctionType.Exp)
                            v = i - 4 * j
                            if v >= 0:
                                nc.vector.tensor_mul(
                                    pt[:], pt[:], mk_sb[:, v * 512:(v + 1) * 512])
                            for t in range(4):
                                if i > 4 * j + t:
                                    continue  # fully-masked block
                                nc.tensor.matmul(
                                    avp[t][:],
                                    pt[:, t * 128:(t + 1) * 128],
                                    v_sb[h][:, i, :],
                                    start=(i == 0), stop=(i == 4 * j + t))
                        for t in range(4):
                            rcp = z_pool.tile([128, 1], F32, tag="rcp")
                            nc.vector.reciprocal(
                                rcp[:], avp[t][:, D_HEAD:D_HEAD + 1])
                            z = z_pool.tile([128, D_HEAD], BF16, tag="z")
                            nc.vector.tensor_scalar_mul(
                                z[:], avp[t][:, 0:D_HEAD], rcp[:])
                            trp = ps_tr.tile([128, 128], BF16, tag="tr")
                            nc.tensor.transpose(trp[:], z[:], ident[:])
                            zt = z_pool.tile([128, D_HEAD], BF16, tag="zt")
                            nc.scalar.copy(zt[:], trp[:])
                            q0 = j * 512 + t * 128
                            nc.sync.dma_start(
                                agin[h * 128:(h + 1) * 128, q0:q0 + 128], zt[:])

            # -------- Phase 3: AllGather + d-sharded output projection ----
            nc.gpsimd.collective_compute(
                "AllGather",
                mybir.AluOpType.bypass,
                replica_groups=GROUPS,
                ins=[agin.ap().opt()],
                outs=[agout.ap().opt()],
            )

            with tc.tile_pool(name="wo", bufs=1) as wo_pool, \
                 tc.tile_pool(name="zg", bufs=N_HEADS) as zg_pool, \
                 tc.tile_pool(name="os", bufs=3) as out_pool, \
                 tc.tile_pool(name="ps_o", bufs=3, space="PSUM") as ps_o:
                wo_sb = wo_pool.tile([128, N_HEADS, QSL], BF16, tag="wo")
                nc.sync.dma_start(
                    wo_sb[:], wo_e.ap().rearrange("(t p) d -> p t d", p=128))

                zg_sb = []
                for tt in range(N_HEADS):
                    zg = zg_pool.tile([128, SEQ], BF16, tag="zg", name=f"zg{tt}")
                    nc.sync.dma_start(
                        zg[:], agout[tt * 128:(tt + 1) * 128, :])
                    zg_sb.append(zg)

                for qt in range(NST):
                    psum = ps_o.tile([128, QSL], F32, tag="po")
                    for tt in range(N_HEADS):
                        nc.tensor.matmul(
                            psum[:],
                            zg_sb[tt][:, qt * 128:(qt + 1) * 128],
                            wo_sb[:, tt, :],
                            start=(tt == 0), stop=(tt == N_HEADS - 1))
                    osb = out_pool.tile([128, QSL], F32, tag="os")
                    nc.scalar.copy(osb[:], psum[:])
                    nc.sync.dma_start(
                        out_e[qt * 128:(qt + 1) * 128, :], osb[:])
    nc.finalize()
    return nc


def _build_masks():
    """mask_v[r, c] = 1 if key position (128*v + r) <= query position c."""
    m = np.zeros((128, 4 * 512), dtype=NPBF16)
    r = np.arange(128)[:, None]
    c = np.arange(512)[None, :]
    for v in range(4):
        m[:, v * 512:(v + 1) * 512] = (c >= 128 * v + r).astype(NPBF16)
    return m


_NC_CACHE = None


def kernel(normalized_resid_pre, W_Q, b_Q, W_K, b_K, W_V, b_V, W_O, b_O):
    global LAST_EXEC_NS, _NC_CACHE
    x = np.asarray(normalized_resid_pre, dtype=np.float32)
    W_Q = np.asarray(W_Q, np.float32); b_Q = np.asarray(b_Q, np.float32)
    W_K = np.asarray(W_K, np.float32); b_K = np.asarray(b_K, np.float32)
    W_V = np.asarray(W_V, np.float32); b_V = np.asarray(b_V, np.float32)
    W_O = np.asarray(W_O, np.float32); b_O = np.asarray(b_O, np.float32)

    mask_m = _build_masks()
    wo_flat = W_O.reshape(N_HEADS * D_HEAD, D_MODEL)
    xt = [np.ascontiguousarray(x[b].T).astype(NPBF16) for b in range(B)]

    in_maps = []
    for c in range(NCORES):
        beta, g = c // 4, c % 4
        hs = slice(HPC * g, HPC * g + HPC)
        wq_m = np.ascontiguousarray(
            W_Q[hs].reshape(HPC * D_MODEL, D_HEAD)).astype(NPBF16)
        wk_m = np.ascontiguousarray(
            W_K[hs].reshape(HPC * D_MODEL, D_HEAD)).astype(NPBF16)
        wv_m = np.ascontiguousarray(
            W_V[hs].transpose(1, 0, 2).reshape(D_MODEL, HPC * D_HEAD)).astype(NPBF16)
        wo_m = np.ascontiguousarray(
            wo_flat[:, QSL * g:QSL * (g + 1)]).astype(NPBF16)
        bq_m = np.ascontiguousarray((b_Q[hs] * SCALE).T).astype(np.float32)
        bk_m = np.ascontiguousarray(b_K[hs].T).astype(np.float32)
        in_maps.append({
            "xt": xt[beta], "wq": wq_m, "wk": wk_m, "wv": wv_m,
            "wo": wo_m, "bq": bq_m, "bk": bk_m, "mk": mask_m,
        })

    if _NC_CACHE is None:
        _NC_CACHE = build_nc()
    nc = _NC_CACHE

    trace = False
    if os.environ.get("BASS_KERNEL_TRACE") == "1":
        try:
            from antenv.axon_hooks import get_axon_ntff_profile_hook
            trace = get_axon_ntff_profile_hook() is not None
        except ImportError:
            trace = False

    res = run_bass_kernel_spmd(nc, in_maps, core_ids=list(range(NCORES)),
                               trace=trace)
    LAST_EXEC_NS = res.exec_time_ns

    # bias correction: softmax rows sum to 1 -> b_V contributes a constant
    # row through W_O; b_O is a plain add.
    corr = b_O + np.einsum("hk,hkd->d", b_V, W_O)

    out = np.empty((B, SEQ, D_MODEL), dtype=np.float32)
    for c in range(NCORES):
        beta, g = c // 4, c % 4
        out[beta, :, QSL * g:QSL * (g + 1)] = (
            res.results[c]["out"] + corr[QSL * g:QSL * (g + 1)])
    return out
